# revision 7
# baseline (speedup 1.0000x reference)
"""Trainium2 Bass kernel: per-batch-row stable partition (facts first, pads last).

For each batch row b: out[b] = sentout[b][order] where order lists positions
with nl_input[b] != 0 first (original order), then positions == 0.

v2 (from trace analysis of the 98.2us baseline):
  - The wall is the DMA-engine/HBM aggregate (~420 GB/s/core mixed): 33.6MB
    (16.8 load + 16.8 scatter) => ~80us of streaming + ~8.6us preamble +
    ~4us postamble.  Baseline losses: scatters idle until 23.7us (nl queued
    behind a 512KB load + slow index chain) and HWDGE load feed capped at
    ~145 GB/s/queue leaving engines 30% idle pre-scatter.
  - nl is host-transposed to [128, NCOL] so the PE transpose disappears;
    nl loads FIRST on the sync queue (lands ~9.3us).
  - Cumsum matmuls run in bf16 (0/1 masks, f32 PSUM accumulate => exact),
    single-pass instead of fp32 double-pump; DVE ops read PSUM directly and
    use scalar_tensor_tensor fusions: dest = A + is_pad*((2C + K) - T) with
    A = l + bL - C and K = 2047 - l a constant tile.
  - One small (2-col) load goes through the gpsimd SWDGE ring FIRST so the
    engines' leftover capacity (~130 GB/s) is not idle before the first
    scatter descriptors exist; all other loads stream on the two HWDGE
    queues (sync+scalar), small units first for an early scatter handoff.
  - Scatter descriptor generation is batched per load unit (up to 512
    descriptors per DMA_INDIRECT; 994ns fixed overhead per call dominates).
  - Scatter->scatter WAW sync deps downgraded to engine-order-only
    (destinations are a permutation => disjoint).
"""

import numpy as np

import concourse.bass as bass
import concourse.mybir as mybir
import concourse.tile as tile
from concourse.bacc import Bacc
from concourse.bass_utils import run_bass_kernel_spmd
from concourse.masks import make_upper_triangular

B, L, D = 16, 2048, 1024
NCORES = 8
BLOC = B // NCORES          # batch rows per core = 2
P = 128                     # SBUF partitions
NCHUNK = L // P             # 16 chunks of 128 rows per batch row
NCOL = BLOC * NCHUNK        # 32 columns in the index layout

# Load-unit plan: (engine, [cols]) in scatter order.  'sp'/'act' are the two
# HWDGE queues; 'ring' goes through the gpsimd SWDGE ring (enqueued before
# any scatter so the ring head feeds engines during the pre-scatter window).
UNITS = [
    ("act", [0]),
    ("sp", [1]),
    ("act", [2]),
    ("sp", [3]),
    ("act", [4, 5]),
    ("sp", [6, 7]),
    ("act", [8, 9, 10, 11]),
    ("sp", [12, 13, 14, 15]),
    ("act", [16, 17, 18, 19]),
    ("sp", [20, 21, 22, 23]),
    ("act", [24, 25, 26, 27]),
    ("sp", [28, 29, 30, 31]),
]

_NC_CACHE = None


def _build_nc(units=None):
    f32 = mybir.dt.float32
    bf16 = mybir.dt.bfloat16
    i32 = mybir.dt.int32
    Op = mybir.AluOpType
    units = units if units is not None else UNITS
    assert sorted(c for _, cols in units for c in cols) == list(range(NCOL))

    nc = Bacc()
    sent = nc.declare_dram_parameter("sent", [BLOC * L, D], f32, isOutput=False)
    # nl arrives host-transposed: nl[p, b*NCHUNK + c] = nl_input[b, c*128 + p]
    nl = nc.declare_dram_parameter("nl", [P, NCOL], i32, isOutput=False)
    out = nc.declare_dram_parameter("out", [BLOC * L, D], f32, isOutput=True)

    with tile.TileContext(nc) as tc:
        with (
            tc.tile_pool(name="const", bufs=1) as cpool,
            tc.tile_pool(name="idx", bufs=1) as ipool,
            tc.tile_pool(name="psum", bufs=2, space="PSUM") as ppool,
            tc.tile_pool(name="d1", bufs=4) as d1pool,
            tc.tile_pool(name="d2", bufs=2) as d2pool,
            tc.tile_pool(name="d4", bufs=6) as d4pool,
        ):
            pools = {1: d1pool, 2: d2pool, 4: d4pool}

            # ---- head-of-queue DMAs: nl first (tiny; index pipeline gate),
            # then the first data units on each HWDGE queue ----
            nl_t = ipool.tile([P, NCOL], i32)
            nc.sync.dma_start(nl_t[:], nl[:])

            dtiles = {}
            for ui, (eng, cols) in enumerate(units):
                if eng == "ring":
                    continue
                K = len(cols)
                dt = pools[K].tile([P, K * D], f32, tag=f"d{K}", name=f"d{K}")
                e = nc.sync if eng == "sp" else nc.scalar
                e.dma_start(
                    dt[:].rearrange("p (g d) -> p g d", g=K),
                    sent[cols[0] * P : (cols[0] + K) * P, :].rearrange(
                        "(g p) d -> p g d", p=P
                    ),
                )
                dtiles[ui] = dt

            # ---- early ring load(s): enqueued on the SWDGE ring before any
            # scatter so engines' leftover capacity is busy from t~9us ----
            for ui, (eng, cols) in enumerate(units):
                if eng != "ring":
                    continue
                K = len(cols)
                dt = pools[K].tile([P, K * D], f32, tag=f"d{K}", name=f"d{K}")
                nc.gpsimd.dma_start(
                    dt[:].rearrange("p (g d) -> p g d", g=K),
                    sent[cols[0] * P : (cols[0] + K) * P, :].rearrange(
                        "(g p) d -> p g d", p=P
                    ),
                )
                dtiles[ui] = dt

            # ---- constants (gpsimd; data-independent, off the nl path) ----
            ut = cpool.tile([P, P], bf16)          # ut[q,p] = 1 iff q <= p
            make_upper_triangular(nc, ut[:], val=1.0, diag=True)
            ones = cpool.tile([P, P], bf16)
            nc.gpsimd.memset(ones[:], 1.0)
            # l(p, j) with j = b*NCHUNK + c  ->  l = p + 128*c ; lfb = l + b*L
            lfb_i = cpool.tile([P, NCOL], i32)
            nc.gpsimd.iota(
                lfb_i[:], [[L, BLOC], [P, NCHUNK]], base=0, channel_multiplier=1
            )
            lfb = cpool.tile([P, NCOL], f32)
            nc.vector.tensor_copy(lfb[:], lfb_i[:])
            # K(p, j) = (L-1) - l  (b-independent)
            kt_i = cpool.tile([P, NCOL], i32)
            nc.gpsimd.iota(
                kt_i[:], [[0, BLOC], [P, NCHUNK]], base=0, channel_multiplier=1
            )
            kt = cpool.tile([P, NCOL], f32)
            nc.vector.tensor_copy(kt[:], kt_i[:])
            nc.vector.tensor_scalar(
                kt[:], kt[:], -1.0, float(L - 1), Op.mult, Op.add
            )

            # ---- index pipeline (DVE + PE + one ACT cast) ----
            # is_pad as f32 and bf16
            ispad = ipool.tile([P, NCOL], f32)
            nc.vector.tensor_scalar(ispad[:], nl_t[:], 0.0, None, Op.is_equal)
            ispad_bf = ipool.tile([P, NCOL], bf16)
            nc.vector.tensor_copy(ispad_bf[:], ispad[:])

            # within-column (partition-dim) inclusive cumsum + column sums
            cw_ps = ppool.tile([P, NCOL], f32)
            nc.tensor.matmul(cw_ps[:], lhsT=ut[:], rhs=ispad_bf[:], start=True, stop=True)
            s_ps = ppool.tile([P, NCOL], f32)
            nc.tensor.matmul(s_ps[:], lhsT=ones[:], rhs=ispad_bf[:], start=True, stop=True)

            # per-b inclusive prefix of column sums along the NCHUNK chunks
            incl = ipool.tile([P, NCOL], f32)
            for b in range(BLOC):
                blk = slice(b * NCHUNK, (b + 1) * NCHUNK)
                nc.vector.tensor_tensor_scan(
                    incl[:, blk], s_ps[:, blk], ispad[:, blk], 0.0, Op.add, Op.bypass
                )
            # C = (incl - s) + cw   (inclusive cumsum of is_pad over l, per b)
            cfull = ipool.tile([P, NCOL], f32)
            nc.vector.tensor_tensor(out=cfull[:], in0=incl[:], in1=s_ps[:], op=Op.subtract)
            nc.vector.tensor_tensor(out=cfull[:], in0=cfull[:], in1=cw_ps[:], op=Op.add)

            # A = lfb - C  (fact destination, incl. batch-row base)
            af = ipool.tile([P, NCOL], f32)
            nc.vector.tensor_tensor(out=af[:], in0=lfb[:], in1=cfull[:], op=Op.subtract)
            # diff0 = 2C + K ; per-b: diffm = (diff0 - T_b) * is_pad
            diff0 = ipool.tile([P, NCOL], f32)
            nc.vector.scalar_tensor_tensor(
                out=diff0[:], in0=cfull[:], scalar=2.0, in1=kt[:],
                op0=Op.mult, op1=Op.add,
            )
            diffm = ipool.tile([P, NCOL], f32)
            for b in range(BLOC):
                blk = slice(b * NCHUNK, (b + 1) * NCHUNK)
                tb = incl[:, (b + 1) * NCHUNK - 1 : (b + 1) * NCHUNK]
                nc.vector.scalar_tensor_tensor(
                    out=diffm[:, blk], in0=diff0[:, blk], scalar=tb,
                    in1=ispad[:, blk], op0=Op.subtract, op1=Op.mult,
                )
            destf = ipool.tile([P, NCOL], f32)
            nc.vector.tensor_tensor(out=destf[:], in0=af[:], in1=diffm[:], op=Op.add)
            dest_all = ipool.tile([P, NCOL], i32)
            nc.vector.tensor_copy(dest_all[:], destf[:])

            # ---- scatters: per-column DMA_INDIRECT (multi-column offset APs
            # misroute every 8th descriptor on HW Q7; single-column is exact
            # and the 994ns/call feed still outpaces the ~420 GB/s drain) ----
            scatter_names = set()
            for ui, (eng, cols) in enumerate(units):
                dt = dtiles[ui]
                for j, c in enumerate(cols):
                    sc = nc.gpsimd.indirect_dma_start(
                        out=out[:],
                        out_offset=bass.IndirectOffsetOnAxis(
                            ap=dest_all[:, c : c + 1], axis=0
                        ),
                        in_=dt[:, j * D : (j + 1) * D],
                        in_offset=None,
                    )
                    # The scatters write pairwise-disjoint row sets of `out`
                    # (dest is a permutation), so WAW completion-waits between
                    # them are spurious; keep engine-order only.
                    mi = sc.ins
                    for dep in mi.sync_dependency_names():
                        if dep in scatter_names:
                            mi.remove_dependency(dep, mybir.DependencyInfo.SYNC_ONLY)
                            mi.add_dependency(dep, mybir.DependencyInfo.NO_SYNC_ONLY)
                    scatter_names.add(mi.name)
    nc.compile()
    return nc


def _get_nc():
    global _NC_CACHE
    if _NC_CACHE is None:
        _NC_CACHE = _build_nc()
    return _NC_CACHE


def _make_in_maps(sentout, nl_input):
    sent = np.ascontiguousarray(np.asarray(sentout, dtype=np.float32)).reshape(
        NCORES, BLOC * L, D
    )
    # host-side transpose of the tiny index tensor:
    # nlT[core, p, b*NCHUNK + c] = nl[core, b, c*128 + p]
    nl = np.asarray(nl_input).astype(np.int32).reshape(NCORES, BLOC, NCHUNK, P)
    nlT = np.ascontiguousarray(nl.transpose(0, 3, 1, 2).reshape(NCORES, P, NCOL))
    return [{"sent": sent[c], "nl": nlT[c]} for c in range(NCORES)]


def run_on_device(sentout, nl_input, **kwargs):
    """Run the Bass kernel; returns (full_output, BassKernelResults)."""
    nc = _get_nc()
    res = run_bass_kernel_spmd(
        nc, _make_in_maps(sentout, nl_input), core_ids=list(range(NCORES)), **kwargs
    )
    outs = [r["out"].reshape(BLOC, L, D) for r in res.results]
    return np.concatenate(outs, axis=0), res


def kernel(sentout, nl_input):
    out, _ = run_on_device(sentout, nl_input)
    return out


# revision 8
# speedup vs baseline: 1.1888x; 1.1888x over previous
"""Trainium2 Bass kernel: per-batch-row stable partition (facts first, pads last).

For each batch row b: out[b] = sentout[b][order] where order lists positions
with nl_input[b] != 0 first (original order), then positions == 0.

v2 (from trace analysis of the 98.2us baseline):
  - The wall is the DMA-engine/HBM aggregate (~420 GB/s/core mixed): 33.6MB
    (16.8 load + 16.8 scatter) => ~80us of streaming + ~8.6us preamble +
    ~4us postamble.  Baseline losses: scatters idle until 23.7us (nl queued
    behind a 512KB load + slow index chain) and HWDGE load feed capped at
    ~145 GB/s/queue leaving engines 30% idle pre-scatter.
  - nl is host-transposed to [128, NCOL] so the PE transpose disappears;
    nl loads FIRST on the sync queue (lands ~9.3us).
  - Cumsum matmuls run in bf16 (0/1 masks, f32 PSUM accumulate => exact),
    single-pass instead of fp32 double-pump; DVE ops read PSUM directly and
    use scalar_tensor_tensor fusions: dest = A + is_pad*((2C + K) - T) with
    A = l + bL - C and K = 2047 - l a constant tile.
  - One small (2-col) load goes through the gpsimd SWDGE ring FIRST so the
    engines' leftover capacity (~130 GB/s) is not idle before the first
    scatter descriptors exist; all other loads stream on the two HWDGE
    queues (sync+scalar), small units first for an early scatter handoff.
  - Scatter descriptor generation is batched per load unit (up to 512
    descriptors per DMA_INDIRECT; 994ns fixed overhead per call dominates).
  - Scatter->scatter WAW sync deps downgraded to engine-order-only
    (destinations are a permutation => disjoint).
"""

import numpy as np

import concourse.bass as bass
import concourse.mybir as mybir
import concourse.tile as tile
from concourse.bacc import Bacc
from concourse.bass_utils import run_bass_kernel_spmd
from concourse.masks import make_upper_triangular

B, L, D = 16, 2048, 1024
NCORES = 8
BLOC = B // NCORES          # batch rows per core = 2
P = 128                     # SBUF partitions
NCHUNK = L // P             # 16 chunks of 128 rows per batch row
NCOL = BLOC * NCHUNK        # 32 columns in the index layout

# Load-unit plan: (engine, [cols]) in scatter order.  'sp'/'act' are the two
# HWDGE queues; 'ring' goes through the gpsimd SWDGE ring (enqueued before
# any scatter so the ring head feeds engines during the pre-scatter window).
UNITS = [
    ("act", [0]),
    ("sp", [1]),
    ("act", [2]),
    ("sp", [3]),
    ("act", [4, 5]),
    ("sp", [6, 7]),
    ("act", [8, 9, 10, 11]),
    ("sp", [12, 13, 14, 15]),
    ("act", [16, 17, 18, 19]),
    ("sp", [20, 21, 22, 23]),
    ("act", [24, 25, 26, 27]),
    ("sp", [28, 29, 30, 31]),
]

_NC_CACHE = None


def _build_nc(units=None):
    f32 = mybir.dt.float32
    bf16 = mybir.dt.bfloat16
    i32 = mybir.dt.int32
    Op = mybir.AluOpType
    units = units if units is not None else UNITS
    assert sorted(c for _, cols in units for c in cols) == list(range(NCOL))

    nc = Bacc()
    sent = nc.declare_dram_parameter("sent", [BLOC * L, D], f32, isOutput=False)
    # nl arrives host-transposed: nl[p, b*NCHUNK + c] = nl_input[b, c*128 + p]
    nl = nc.declare_dram_parameter("nl", [P, NCOL], i32, isOutput=False)
    out = nc.declare_dram_parameter("out", [BLOC * L, D], f32, isOutput=True)

    with tile.TileContext(nc) as tc:
        with (
            tc.tile_pool(name="const", bufs=1) as cpool,
            tc.tile_pool(name="idx", bufs=1) as ipool,
            tc.tile_pool(name="psum", bufs=2, space="PSUM") as ppool,
            tc.tile_pool(name="d1", bufs=2) as d1pool,
            tc.tile_pool(name="d2", bufs=3) as d2pool,
            tc.tile_pool(name="d4", bufs=6) as d4pool,
        ):
            pools = {1: d1pool, 2: d2pool, 4: d4pool}

            # ---- head-of-queue DMAs: nl first (tiny; index pipeline gate),
            # then the first data units on each HWDGE queue ----
            nl_t = ipool.tile([P, NCOL], i32)
            nc.sync.dma_start(nl_t[:], nl[:])

            dtiles = {}
            for ui, (eng, cols) in enumerate(units):
                if eng == "ring":
                    continue
                K = len(cols)
                dt = pools[K].tile([P, K * D], f32, tag=f"d{K}", name=f"d{K}")
                e = nc.sync if eng == "sp" else nc.scalar
                e.dma_start(
                    dt[:].rearrange("p (g d) -> p g d", g=K),
                    sent[cols[0] * P : (cols[0] + K) * P, :].rearrange(
                        "(g p) d -> p g d", p=P
                    ),
                )
                dtiles[ui] = dt

            # ---- early ring load(s): enqueued on the SWDGE ring before any
            # scatter so engines' leftover capacity is busy from t~9us ----
            for ui, (eng, cols) in enumerate(units):
                if eng != "ring":
                    continue
                K = len(cols)
                dt = pools[K].tile([P, K * D], f32, tag=f"d{K}", name=f"d{K}")
                nc.gpsimd.dma_start(
                    dt[:].rearrange("p (g d) -> p g d", g=K),
                    sent[cols[0] * P : (cols[0] + K) * P, :].rearrange(
                        "(g p) d -> p g d", p=P
                    ),
                )
                dtiles[ui] = dt

            # ---- constants (gpsimd; data-independent, off the nl path) ----
            ut = cpool.tile([P, P], bf16)          # ut[q,p] = 1 iff q <= p
            make_upper_triangular(nc, ut[:], val=1.0, diag=True)
            ones = cpool.tile([P, P], bf16)
            nc.gpsimd.memset(ones[:], 1.0)
            # l(p, j) with j = b*NCHUNK + c  ->  l = p + 128*c ; lfb = l + b*L
            lfb_i = cpool.tile([P, NCOL], i32)
            nc.gpsimd.iota(
                lfb_i[:], [[L, BLOC], [P, NCHUNK]], base=0, channel_multiplier=1
            )
            lfb = cpool.tile([P, NCOL], f32)
            nc.vector.tensor_copy(lfb[:], lfb_i[:])
            # K(p, j) = (L-1) - l  (b-independent)
            kt_i = cpool.tile([P, NCOL], i32)
            nc.gpsimd.iota(
                kt_i[:], [[0, BLOC], [P, NCHUNK]], base=0, channel_multiplier=1
            )
            kt = cpool.tile([P, NCOL], f32)
            nc.vector.tensor_copy(kt[:], kt_i[:])
            nc.vector.tensor_scalar(
                kt[:], kt[:], -1.0, float(L - 1), Op.mult, Op.add
            )

            # ---- index pipeline (DVE + PE + one ACT cast) ----
            # is_pad as f32 and bf16
            ispad = ipool.tile([P, NCOL], f32)
            nc.vector.tensor_scalar(ispad[:], nl_t[:], 0.0, None, Op.is_equal)
            ispad_bf = ipool.tile([P, NCOL], bf16)
            nc.vector.tensor_copy(ispad_bf[:], ispad[:])

            # within-column (partition-dim) inclusive cumsum + column sums
            cw_ps = ppool.tile([P, NCOL], f32)
            nc.tensor.matmul(cw_ps[:], lhsT=ut[:], rhs=ispad_bf[:], start=True, stop=True)
            s_ps = ppool.tile([P, NCOL], f32)
            nc.tensor.matmul(s_ps[:], lhsT=ones[:], rhs=ispad_bf[:], start=True, stop=True)

            # per-b inclusive prefix of column sums along the NCHUNK chunks
            incl = ipool.tile([P, NCOL], f32)
            for b in range(BLOC):
                blk = slice(b * NCHUNK, (b + 1) * NCHUNK)
                nc.vector.tensor_tensor_scan(
                    incl[:, blk], s_ps[:, blk], ispad[:, blk], 0.0, Op.add, Op.bypass
                )
            # C = (incl - s) + cw   (inclusive cumsum of is_pad over l, per b)
            cfull = ipool.tile([P, NCOL], f32)
            nc.vector.tensor_tensor(out=cfull[:], in0=incl[:], in1=s_ps[:], op=Op.subtract)
            nc.vector.tensor_tensor(out=cfull[:], in0=cfull[:], in1=cw_ps[:], op=Op.add)

            # A = lfb - C  (fact destination, incl. batch-row base)
            af = ipool.tile([P, NCOL], f32)
            nc.vector.tensor_tensor(out=af[:], in0=lfb[:], in1=cfull[:], op=Op.subtract)
            # diff0 = 2C + K ; per-b: diffm = (diff0 - T_b) * is_pad
            diff0 = ipool.tile([P, NCOL], f32)
            nc.vector.scalar_tensor_tensor(
                out=diff0[:], in0=cfull[:], scalar=2.0, in1=kt[:],
                op0=Op.mult, op1=Op.add,
            )
            diffm = ipool.tile([P, NCOL], f32)
            for b in range(BLOC):
                blk = slice(b * NCHUNK, (b + 1) * NCHUNK)
                tb = incl[:, (b + 1) * NCHUNK - 1 : (b + 1) * NCHUNK]
                nc.vector.scalar_tensor_tensor(
                    out=diffm[:, blk], in0=diff0[:, blk], scalar=tb,
                    in1=ispad[:, blk], op0=Op.subtract, op1=Op.mult,
                )
            destf = ipool.tile([P, NCOL], f32)
            nc.vector.tensor_tensor(out=destf[:], in0=af[:], in1=diffm[:], op=Op.add)
            dest_all = ipool.tile([P, NCOL], i32)
            nc.vector.tensor_copy(dest_all[:], destf[:])

            # ---- scatters: per-column DMA_INDIRECT (multi-column offset APs
            # misroute every 8th descriptor on HW Q7; single-column is exact
            # and the 994ns/call feed still outpaces the ~420 GB/s drain) ----
            scatter_names = set()
            for ui, (eng, cols) in enumerate(units):
                dt = dtiles[ui]
                for j, c in enumerate(cols):
                    sc = nc.gpsimd.indirect_dma_start(
                        out=out[:],
                        out_offset=bass.IndirectOffsetOnAxis(
                            ap=dest_all[:, c : c + 1], axis=0
                        ),
                        in_=dt[:, j * D : (j + 1) * D],
                        in_offset=None,
                    )
                    # The scatters write pairwise-disjoint row sets of `out`
                    # (dest is a permutation), so WAW completion-waits between
                    # them are spurious; keep engine-order only.
                    mi = sc.ins
                    for dep in mi.sync_dependency_names():
                        if dep in scatter_names:
                            mi.remove_dependency(dep, mybir.DependencyInfo.SYNC_ONLY)
                            mi.add_dependency(dep, mybir.DependencyInfo.NO_SYNC_ONLY)
                    scatter_names.add(mi.name)
    nc.compile()
    return nc


def _get_nc():
    global _NC_CACHE
    if _NC_CACHE is None:
        _NC_CACHE = _build_nc()
    return _NC_CACHE


def _make_in_maps(sentout, nl_input):
    sent = np.ascontiguousarray(np.asarray(sentout, dtype=np.float32)).reshape(
        NCORES, BLOC * L, D
    )
    # host-side transpose of the tiny index tensor:
    # nlT[core, p, b*NCHUNK + c] = nl[core, b, c*128 + p]
    nl = np.asarray(nl_input).astype(np.int32).reshape(NCORES, BLOC, NCHUNK, P)
    nlT = np.ascontiguousarray(nl.transpose(0, 3, 1, 2).reshape(NCORES, P, NCOL))
    return [{"sent": sent[c], "nl": nlT[c]} for c in range(NCORES)]


def run_on_device(sentout, nl_input, **kwargs):
    """Run the Bass kernel; returns (full_output, BassKernelResults)."""
    nc = _get_nc()
    res = run_bass_kernel_spmd(
        nc, _make_in_maps(sentout, nl_input), core_ids=list(range(NCORES)), **kwargs
    )
    outs = [r["out"].reshape(BLOC, L, D) for r in res.results]
    return np.concatenate(outs, axis=0), res


def kernel(sentout, nl_input):
    out, _ = run_on_device(sentout, nl_input)
    return out


# revision 10
# speedup vs baseline: 1.1997x; 1.0091x over previous
"""Trainium2 Bass kernel: per-batch-row stable partition (facts first, pads last).

For each batch row b: out[b] = sentout[b][order] where order lists positions
with nl_input[b] != 0 first (original order), then positions == 0.

v2 (from trace analysis of the 98.2us baseline):
  - The wall is the DMA-engine/HBM aggregate (~420 GB/s/core mixed): 33.6MB
    (16.8 load + 16.8 scatter) => ~80us of streaming + ~8.6us preamble +
    ~4us postamble.  Baseline losses: scatters idle until 23.7us (nl queued
    behind a 512KB load + slow index chain) and HWDGE load feed capped at
    ~145 GB/s/queue leaving engines 30% idle pre-scatter.
  - nl is host-transposed to [128, NCOL] so the PE transpose disappears;
    nl loads FIRST on the sync queue (lands ~9.3us).
  - Cumsum matmuls run in bf16 (0/1 masks, f32 PSUM accumulate => exact),
    single-pass instead of fp32 double-pump; DVE ops read PSUM directly and
    use scalar_tensor_tensor fusions: dest = A + is_pad*((2C + K) - T) with
    A = l + bL - C and K = 2047 - l a constant tile.
  - One small (2-col) load goes through the gpsimd SWDGE ring FIRST so the
    engines' leftover capacity (~130 GB/s) is not idle before the first
    scatter descriptors exist; all other loads stream on the two HWDGE
    queues (sync+scalar), small units first for an early scatter handoff.
  - Scatter descriptor generation is batched per load unit (up to 512
    descriptors per DMA_INDIRECT; 994ns fixed overhead per call dominates).
  - Scatter->scatter WAW sync deps downgraded to engine-order-only
    (destinations are a permutation => disjoint).
"""

import numpy as np

import concourse.bass as bass
import concourse.mybir as mybir
import concourse.tile as tile
from concourse.bacc import Bacc
from concourse.bass_utils import run_bass_kernel_spmd
from concourse.masks import make_upper_triangular

B, L, D = 16, 2048, 1024
NCORES = 8
BLOC = B // NCORES          # batch rows per core = 2
P = 128                     # SBUF partitions
NCHUNK = L // P             # 16 chunks of 128 rows per batch row
NCOL = BLOC * NCHUNK        # 32 columns in the index layout

# Load-unit plan: (engine, [cols]) in scatter order.  'sp'/'act' are the two
# HWDGE queues; 'ring' goes through the gpsimd SWDGE ring (enqueued before
# any scatter so the ring head feeds engines during the pre-scatter window).
UNITS = [
    ("act", [0]),
    ("sp", [1]),
    ("act", [2]),
    ("sp", [3]),
    ("act", [4, 5]),
    ("sp", [6, 7]),
    ("act", [8, 9, 10, 11]),
    ("sp", [12, 13, 14, 15]),
    ("act", [16, 17, 18, 19]),
    ("sp", [20, 21, 22, 23]),
    ("act", [24, 25, 26, 27]),
    ("sp", [28, 29, 30, 31]),
]

_NC_CACHE = None


def _build_nc(units=None):
    f32 = mybir.dt.float32
    bf16 = mybir.dt.bfloat16
    i32 = mybir.dt.int32
    Op = mybir.AluOpType
    units = units if units is not None else UNITS
    assert sorted(c for _, cols in units for c in cols) == list(range(NCOL))

    nc = Bacc()
    sent = nc.declare_dram_parameter("sent", [BLOC * L, D], f32, isOutput=False)
    # nl arrives host-transposed: nl[p, b*NCHUNK + c] = nl_input[b, c*128 + p]
    nl = nc.declare_dram_parameter("nl", [P, NCOL], i32, isOutput=False)
    out = nc.declare_dram_parameter("out", [BLOC * L, D], f32, isOutput=True)

    with tile.TileContext(nc) as tc:
        with (
            tc.tile_pool(name="const", bufs=1) as cpool,
            tc.tile_pool(name="idx", bufs=1) as ipool,
            tc.tile_pool(name="psum", bufs=2, space="PSUM") as ppool,
            tc.tile_pool(name="d1", bufs=4) as d1pool,
            tc.tile_pool(name="d2", bufs=2) as d2pool,
            tc.tile_pool(name="d4", bufs=6) as d4pool,
        ):
            pools = {1: d1pool, 2: d2pool, 4: d4pool}

            # ---- head-of-queue DMAs: nl first (tiny; index pipeline gate),
            # then the first data units on each HWDGE queue ----
            nl_t = ipool.tile([P, NCOL], i32)
            nc.sync.dma_start(nl_t[:], nl[:])

            dtiles = {}
            for ui, (eng, cols) in enumerate(units):
                if eng == "ring":
                    continue
                K = len(cols)
                dt = pools[K].tile([P, K * D], f32, tag=f"d{K}", name=f"d{K}")
                e = nc.sync if eng == "sp" else nc.scalar
                e.dma_start(
                    dt[:].rearrange("p (g d) -> p g d", g=K),
                    sent[cols[0] * P : (cols[0] + K) * P, :].rearrange(
                        "(g p) d -> p g d", p=P
                    ),
                )
                dtiles[ui] = dt

            # ---- early ring load(s): enqueued on the SWDGE ring before any
            # scatter so engines' leftover capacity is busy from t~9us ----
            for ui, (eng, cols) in enumerate(units):
                if eng != "ring":
                    continue
                K = len(cols)
                dt = pools[K].tile([P, K * D], f32, tag=f"d{K}", name=f"d{K}")
                nc.gpsimd.dma_start(
                    dt[:].rearrange("p (g d) -> p g d", g=K),
                    sent[cols[0] * P : (cols[0] + K) * P, :].rearrange(
                        "(g p) d -> p g d", p=P
                    ),
                )
                dtiles[ui] = dt

            # ---- constants (gpsimd; data-independent, off the nl path) ----
            ut = cpool.tile([P, P], bf16)          # ut[q,p] = 1 iff q <= p
            make_upper_triangular(nc, ut[:], val=1.0, diag=True)
            ones = cpool.tile([P, P], bf16)
            nc.gpsimd.memset(ones[:], 1.0)
            # l(p, j) with j = b*NCHUNK + c  ->  l = p + 128*c ; lfb = l + b*L
            lfb_i = cpool.tile([P, NCOL], i32)
            nc.gpsimd.iota(
                lfb_i[:], [[L, BLOC], [P, NCHUNK]], base=0, channel_multiplier=1
            )
            lfb = cpool.tile([P, NCOL], f32)
            nc.vector.tensor_copy(lfb[:], lfb_i[:])
            # K(p, j) = (L-1) - l  (b-independent)
            kt_i = cpool.tile([P, NCOL], i32)
            nc.gpsimd.iota(
                kt_i[:], [[0, BLOC], [P, NCHUNK]], base=0, channel_multiplier=1
            )
            kt = cpool.tile([P, NCOL], f32)
            nc.vector.tensor_copy(kt[:], kt_i[:])
            nc.vector.tensor_scalar(
                kt[:], kt[:], -1.0, float(L - 1), Op.mult, Op.add
            )

            # ---- index pipeline (DVE + PE + one ACT cast) ----
            # is_pad as f32 and bf16
            ispad = ipool.tile([P, NCOL], f32)
            nc.vector.tensor_scalar(ispad[:], nl_t[:], 0.0, None, Op.is_equal)
            ispad_bf = ipool.tile([P, NCOL], bf16)
            nc.vector.tensor_copy(ispad_bf[:], ispad[:])

            # within-column (partition-dim) inclusive cumsum + column sums
            cw_ps = ppool.tile([P, NCOL], f32)
            nc.tensor.matmul(cw_ps[:], lhsT=ut[:], rhs=ispad_bf[:], start=True, stop=True)
            s_ps = ppool.tile([P, NCOL], f32)
            nc.tensor.matmul(s_ps[:], lhsT=ones[:], rhs=ispad_bf[:], start=True, stop=True)

            # per-b inclusive prefix of column sums along the NCHUNK chunks
            incl = ipool.tile([P, NCOL], f32)
            for b in range(BLOC):
                blk = slice(b * NCHUNK, (b + 1) * NCHUNK)
                nc.vector.tensor_tensor_scan(
                    incl[:, blk], s_ps[:, blk], ispad[:, blk], 0.0, Op.add, Op.bypass
                )
            # C = (incl - s) + cw   (inclusive cumsum of is_pad over l, per b)
            cfull = ipool.tile([P, NCOL], f32)
            nc.vector.tensor_tensor(out=cfull[:], in0=incl[:], in1=s_ps[:], op=Op.subtract)
            nc.vector.tensor_tensor(out=cfull[:], in0=cfull[:], in1=cw_ps[:], op=Op.add)

            # A = lfb - C  (fact destination, incl. batch-row base)
            af = ipool.tile([P, NCOL], f32)
            nc.vector.tensor_tensor(out=af[:], in0=lfb[:], in1=cfull[:], op=Op.subtract)
            # diff0 = 2C + K ; per-b: diffm = (diff0 - T_b) * is_pad
            diff0 = ipool.tile([P, NCOL], f32)
            nc.vector.scalar_tensor_tensor(
                out=diff0[:], in0=cfull[:], scalar=2.0, in1=kt[:],
                op0=Op.mult, op1=Op.add,
            )
            diffm = ipool.tile([P, NCOL], f32)
            for b in range(BLOC):
                blk = slice(b * NCHUNK, (b + 1) * NCHUNK)
                tb = incl[:, (b + 1) * NCHUNK - 1 : (b + 1) * NCHUNK]
                nc.vector.scalar_tensor_tensor(
                    out=diffm[:, blk], in0=diff0[:, blk], scalar=tb,
                    in1=ispad[:, blk], op0=Op.subtract, op1=Op.mult,
                )
            destf = ipool.tile([P, NCOL], f32)
            nc.vector.tensor_tensor(out=destf[:], in0=af[:], in1=diffm[:], op=Op.add)
            dest_all = ipool.tile([P, NCOL], i32)
            nc.vector.tensor_copy(dest_all[:], destf[:])

            # ---- scatters: per-column DMA_INDIRECT (multi-column offset APs
            # misroute every 8th descriptor on HW Q7; single-column is exact
            # and the 994ns/call feed still outpaces the ~420 GB/s drain) ----
            scatter_names = set()
            for ui, (eng, cols) in enumerate(units):
                dt = dtiles[ui]
                for j, c in enumerate(cols):
                    sc = nc.gpsimd.indirect_dma_start(
                        out=out[:],
                        out_offset=bass.IndirectOffsetOnAxis(
                            ap=dest_all[:, c : c + 1], axis=0
                        ),
                        in_=dt[:, j * D : (j + 1) * D],
                        in_offset=None,
                    )
                    # The scatters write pairwise-disjoint row sets of `out`
                    # (dest is a permutation), so WAW completion-waits between
                    # them are spurious; keep engine-order only.
                    mi = sc.ins
                    for dep in mi.sync_dependency_names():
                        if dep in scatter_names:
                            mi.remove_dependency(dep, mybir.DependencyInfo.SYNC_ONLY)
                            mi.add_dependency(dep, mybir.DependencyInfo.NO_SYNC_ONLY)
                    scatter_names.add(mi.name)
    nc.compile()
    return nc


def _get_nc():
    global _NC_CACHE
    if _NC_CACHE is None:
        _NC_CACHE = _build_nc()
    return _NC_CACHE


def _make_in_maps(sentout, nl_input):
    sent = np.ascontiguousarray(np.asarray(sentout, dtype=np.float32)).reshape(
        NCORES, BLOC * L, D
    )
    # host-side transpose of the tiny index tensor:
    # nlT[core, p, b*NCHUNK + c] = nl[core, b, c*128 + p]
    nl = np.asarray(nl_input).astype(np.int32).reshape(NCORES, BLOC, NCHUNK, P)
    nlT = np.ascontiguousarray(nl.transpose(0, 3, 1, 2).reshape(NCORES, P, NCOL))
    return [{"sent": sent[c], "nl": nlT[c]} for c in range(NCORES)]


def run_on_device(sentout, nl_input, **kwargs):
    """Run the Bass kernel; returns (full_output, BassKernelResults)."""
    nc = _get_nc()
    res = run_bass_kernel_spmd(
        nc, _make_in_maps(sentout, nl_input), core_ids=list(range(NCORES)), **kwargs
    )
    outs = [r["out"].reshape(BLOC, L, D) for r in res.results]
    return np.concatenate(outs, axis=0), res


def kernel(sentout, nl_input):
    out, _ = run_on_device(sentout, nl_input)
    return out
